# revision 1
# baseline (speedup 1.0000x reference)
"""Trainium2 Bass kernel for CausalTensionGraphLayer.

Math (reference factorization):
  a   = x @ w1[:D] + b1         [T, H]   (H = D/2)
  c   = x @ w1[D:]              [T, H]
  vzb = x @ wv_w + wv_b         [T, D]
  hid_w  = silu(a[t] + c[t-w-1])               (c term is 0 when t-w-1 < 0)
  tau_w  = sigmoid(hid_w @ w2 + b2)
  msg[t] = sum_w tau_w[t] * vzb[t-w-1]         (vzb -> wv_b when t-w-1 < 0)
  y      = x @ merge_w[:D] + msg @ merge_w[D:] + merge_b
  out    = LayerNorm(y) * gamma + beta

Neighbor gathers are row shifts of x, so with zero rows prepended for the
out-of-range halo the same compute path reproduces the reference exactly
(zero x rows give c = 0 and vzb = wv_b).

Sharding: data-parallel over the B*T = 8192 token rows, 1024 own tokens per
core plus a 4-row halo (zeros at batch boundaries, neighbor rows otherwise).
No collectives. Host pre-casts x/weights to bf16 and pre-transposes x so the
device works feature-major (tokens on the free axis -> shifts are free-dim
offsets).

Schedule: phase A (a+c, all token quarters) needs only x/w1 so the PE starts
while wv and the merge weights are still streaming in; phase B (vzb) overlaps
the merge-weight loads; phases C (gating) and D (merge+LN) run per quarter.
Input DMAs are split across the two HWDGE queues (sync, scalar) in the exact
order the PE consumes them.
"""

from contextlib import ExitStack

import numpy as np
import ml_dtypes

import concourse.bass as bass
import concourse.bacc as bacc
import concourse.tile as tile
from concourse import mybir
from concourse.bass_utils import run_bass_kernel_spmd

BF16 = ml_dtypes.bfloat16

B, T, D = 2, 4096, 1024
H = D // 2
W = 4
EPS = 1e-5
NCORES = 8
NTOK = (B * T) // NCORES          # 1024 own tokens per core
HALO = W                          # 4
GRID = NTOK + HALO                # 1028 (halo + own)
NQ = 4                            # token quarters per core
QT = NTOK // NQ                   # 256 own tokens per quarter
QG = QT + HALO                    # 260: shifted-grid cols per quarter
KD = D // 128                     # 8 K-chunks over D
MH = H // 128                     # 4 M-tiles over H
MD = D // 128                     # 8 M-tiles over D
NT = QT // 128                    # 2 token tiles per quarter

FP32 = mybir.dt.float32
I32 = mybir.dt.int32
BF = mybir.dt.bfloat16
AF = mybir.ActivationFunctionType
ALU = mybir.AluOpType
AX = mybir.AxisListType


def build_nc(use_gamma_beta: bool, use_merge_b: bool):
    nc = bacc.Bacc(None, target_bir_lowering=False)

    xT = nc.dram_tensor("xT", [D, GRID], BF, kind="ExternalInput")
    w1a = nc.dram_tensor("w1a", [D, H], BF, kind="ExternalInput")
    w1c = nc.dram_tensor("w1c", [D, H], BF, kind="ExternalInput")
    wv = nc.dram_tensor("wv", [D, D], BF, kind="ExternalInput")
    m1 = nc.dram_tensor("m1", [D, D], BF, kind="ExternalInput")
    m2 = nc.dram_tensor("m2", [D, D], BF, kind="ExternalInput")
    w2rep = nc.dram_tensor("w2rep", [H, 128], BF, kind="ExternalInput")
    b1r = nc.dram_tensor("b1r", [128, MH], FP32, kind="ExternalInput")
    wvbr = nc.dram_tensor("wvbr", [128, MD], FP32, kind="ExternalInput")
    b2r = nc.dram_tensor("b2r", [128, 1], FP32, kind="ExternalInput")
    if use_gamma_beta:
        gam = nc.dram_tensor("gam", [1, D], FP32, kind="ExternalInput")
        bet = nc.dram_tensor("bet", [1, D], FP32, kind="ExternalInput")
    if use_merge_b:
        mbt = nc.dram_tensor("mbt", [1, D], FP32, kind="ExternalInput")
    y = nc.dram_tensor("y", [NTOK, D], FP32, kind="ExternalOutput")

    with tile.TileContext(nc) as tc, ExitStack() as ctx:
        persist = ctx.enter_context(tc.tile_pool(name="persist", bufs=1))
        abpool = ctx.enter_context(tc.tile_pool(name="abpool", bufs=NQ))
        qpool = ctx.enter_context(tc.tile_pool(name="qpool", bufs=2))
        mpool = ctx.enter_context(tc.tile_pool(name="mpool", bufs=4))
        mpool2 = ctx.enter_context(tc.tile_pool(name="mpool2", bufs=2))
        opool = ctx.enter_context(tc.tile_pool(name="opool", bufs=3))
        ps_acc = ctx.enter_context(tc.tile_pool(name="ps_acc", bufs=4, space="PSUM"))
        ps_log = ctx.enter_context(tc.tile_pool(name="ps_log", bufs=1, space="PSUM"))
        ps_y = ctx.enter_context(tc.tile_pool(name="ps_y", bufs=3, space="PSUM"))

        # ---- persistent loads, just-in-time order across both queues ----
        xT_sb = persist.tile([128, KD, GRID], BF, tag="xT")
        w1a_sb = persist.tile([128, KD, H], BF, tag="w1a")
        w1c_sb = persist.tile([128, KD, H], BF, tag="w1c")
        w2rep_sb = persist.tile([128, MH, 128], BF, tag="w2rep")
        wv_sb = persist.tile([128, KD, D], BF, tag="wv")
        m1_sb = persist.tile([128, KD, D], BF, tag="m1")
        m2_sb = persist.tile([128, KD, D], BF, tag="m2")
        b1_sb = persist.tile([128, MH], FP32, tag="b1")
        wvb_sb = persist.tile([128, MD], FP32, tag="wvb")
        b2_sb = persist.tile([128, 1], FP32, tag="b2")
        xT_r = xT.rearrange("(n p) t -> p n t", p=128)
        w1a_r = w1a.rearrange("(n p) m -> p n m", p=128)
        w1c_r = w1c.rearrange("(n p) m -> p n m", p=128)
        w2_r = w2rep.rearrange("(n p) m -> p n m", p=128)
        wv_r = wv.rearrange("(n p) m -> p n m", p=128)
        m1_r = m1.rearrange("(n p) m -> p n m", p=128)
        m2_r = m2.rearrange("(n p) m -> p n m", p=128)
        Q1 = QT + HALO
        # Greedy per-queue byte balancing in PE-consumption order:
        #   sync:   xTq0 | w1c | wv[0:4] | xTq3 | m1
        #   scalar: w1a | xTq1 | wv[4:8] | xTq2 | w2rep+biases | m2
        nc.sync.dma_start(out=xT_sb[:, :, 0:Q1], in_=xT_r[:, :, 0:Q1])
        for mc in range(MH):  # w1a col-chunks so a(q0, m=0) unblocks early
            nc.scalar.dma_start(
                out=w1a_sb[:, :, mc * 128:(mc + 1) * 128],
                in_=w1a_r[:, :, mc * 128:(mc + 1) * 128],
            )
        nc.scalar.dma_start(out=b1_sb, in_=b1r[:, :])
        for mc in range(MH):
            nc.sync.dma_start(
                out=w1c_sb[:, :, mc * 128:(mc + 1) * 128],
                in_=w1c_r[:, :, mc * 128:(mc + 1) * 128],
            )
        nc.scalar.dma_start(
            out=xT_sb[:, :, Q1:Q1 + QT], in_=xT_r[:, :, Q1:Q1 + QT]
        )
        nc.scalar.dma_start(out=wvb_sb, in_=wvbr[:, :])
        for mc in range(MD):
            eng = nc.sync if mc < 4 else nc.scalar
            eng.dma_start(
                out=wv_sb[:, :, mc * 128:(mc + 1) * 128],
                in_=wv_r[:, :, mc * 128:(mc + 1) * 128],
            )
        nc.scalar.dma_start(
            out=xT_sb[:, :, Q1 + QT:Q1 + 2 * QT],
            in_=xT_r[:, :, Q1 + QT:Q1 + 2 * QT],
        )
        nc.sync.dma_start(
            out=xT_sb[:, :, Q1 + 2 * QT:GRID], in_=xT_r[:, :, Q1 + 2 * QT:GRID]
        )
        nc.scalar.dma_start(out=w2rep_sb[:, :, :], in_=w2_r[:, :, :])
        nc.scalar.dma_start(out=b2_sb, in_=b2r[:, :])
        # merge weights last (first needed after phases A+B)
        nc.sync.dma_start(out=m1_sb[:, :, 0:512], in_=m1_r[:, :, 0:512])
        nc.scalar.dma_start(out=m2_sb[:, :, 0:512], in_=m2_r[:, :, 0:512])
        nc.sync.dma_start(out=m1_sb[:, :, 512:D], in_=m1_r[:, :, 512:D])
        nc.scalar.dma_start(out=m2_sb[:, :, 512:D], in_=m2_r[:, :, 512:D])
        magic_sb = persist.tile([128, 1], I32, tag="magic")
        nc.vector.memset(magic_sb, 0x5F3759DF)
        one_i = persist.tile([128, 1], I32, tag="onei")
        nc.vector.memset(one_i, 1)
        if use_gamma_beta:
            gam_sb = persist.tile([128, D], FP32, tag="gam")
            nc.sync.dma_start(out=gam_sb, in_=gam.partition_broadcast(128))
            bet_sb = persist.tile([128, D], FP32, tag="bet")
            nc.sync.dma_start(out=bet_sb, in_=bet.partition_broadcast(128))
        if use_merge_b:
            mb_sb = persist.tile([128, D], FP32, tag="mb")
            nc.sync.dma_start(out=mb_sb, in_=mbt.partition_broadcast(128))

        # ---- phase A: a (own grid) and c (shifted grid), all quarters ----
        aqs, cqs, vzqs = [], [], []
        for q in range(NQ):
            g0 = q * QT
            aq = abpool.tile([128, MH, QT], BF, tag="aq")
            aqs.append(aq)
            cq = abpool.tile([128, MH, QG], BF, tag="cq")
            cqs.append(cq)
            for m in range(MH):
                ps = ps_acc.tile([128, QT], FP32, tag="acc")
                for k in range(KD):
                    nc.tensor.matmul(
                        ps,
                        w1a_sb[:, k, m * 128:(m + 1) * 128],
                        xT_sb[:, k, g0 + HALO:g0 + HALO + QT],
                        start=(k == 0),
                        stop=(k == KD - 1),
                    )
                nc.scalar.activation(
                    out=aq[:, m, :], in_=ps, func=AF.Identity,
                    bias=b1_sb[:, m:m + 1], scale=1.0,
                )
            for m in range(MH):
                ps = ps_acc.tile([128, QG], FP32, tag="acc")
                for k in range(KD):
                    nc.tensor.matmul(
                        ps,
                        w1c_sb[:, k, m * 128:(m + 1) * 128],
                        xT_sb[:, k, g0:g0 + QG],
                        start=(k == 0),
                        stop=(k == KD - 1),
                    )
                nc.scalar.copy(out=cq[:, m, :], in_=ps)
        # ---- phase B: vzb (shifted grid), all quarters -------------------
        for q in range(NQ):
            g0 = q * QT
            vzq = abpool.tile([128, MD, QG], BF, tag="vzq")
            vzqs.append(vzq)
            for m in range(MD):
                ps = ps_acc.tile([128, QG], FP32, tag="acc")
                for k in range(KD):
                    nc.tensor.matmul(
                        ps,
                        wv_sb[:, k, m * 128:(m + 1) * 128],
                        xT_sb[:, k, g0:g0 + QG],
                        start=(k == 0),
                        stop=(k == KD - 1),
                    )
                nc.scalar.activation(
                    out=vzq[:, m, :], in_=ps, func=AF.Identity,
                    bias=wvb_sb[:, m:m + 1], scale=1.0,
                )
        # ---- phase C: gating (hid -> tau -> msg), per quarter ------------
        # silu(z) = z * sigmoid(z) keeps ScalarE in one activation-table set
        # for the whole kernel (silu/sqrt live in different sets; a switch
        # costs ~2.7us). tau comes out of its matmul pre-broadcast across
        # partitions because w2 is replicated over all 128 PE columns.
        msgqs = []
        for q in range(NQ):
            aq, cq, vzq = aqs[q], cqs[q], vzqs[q]
            tauq = qpool.tile([128, W, QT], BF, tag="tauq")
            for p in range(W // 2):
                hs = mpool2.tile([128, MH, 2, QT], BF, tag="hs")
                for wi in range(2):
                    w = 2 * p + wi
                    o = HALO - 1 - w
                    nc.vector.tensor_add(
                        hs[:, :, wi, :], aq, cq[:, :, o:o + QT]
                    )
                sg = mpool2.tile([128, MH, 2, QT], BF, tag="sg")
                nc.scalar.activation(out=sg, in_=hs, func=AF.Sigmoid)
                hss = mpool2.tile([128, MH, 2, QT], BF, tag="hids")
                nc.vector.tensor_mul(hss, hs, sg)
                pl = ps_log.tile([128, 2 * QT], FP32, tag="logit")
                for k in range(MH):
                    nc.tensor.matmul(
                        pl,
                        w2rep_sb[:, k, :],
                        hss[:, k, :, :],
                        start=(k == 0),
                        stop=(k == MH - 1),
                    )
                nc.scalar.activation(
                    out=tauq[:, 2 * p:2 * p + 2, :],
                    in_=pl.rearrange("p (a b) -> p a b", a=2),
                    func=AF.Sigmoid,
                    bias=b2_sb[:, 0:1], scale=1.0,
                )
            # msg = sum_w tau_w * shift(vzb, w+1): fused 3D bf16 ops with tau
            # broadcast over the 8 d-tiles via a step-0 mid dimension.
            msgq = qpool.tile([128, MD, QT], BF, tag="msgq")
            msgqs.append(msgq)

            def tau_b(w, tauq=tauq):
                s = tauq[:, w, :]
                return bass.AP(
                    tensor=s.tensor, offset=s.offset,
                    ap=[s.ap[0], [0, MD], s.ap[1]],
                )

            pw = []
            for w in range(W):
                o = HALO - 1 - w
                pt = mpool.tile([128, MD, QT], BF, tag="pw")
                nc.vector.tensor_mul(pt, tau_b(w), vzq[:, :, o:o + QT])
                pw.append(pt)
                if w == 1:
                    m01 = mpool.tile([128, MD, QT], BF, tag="pw")
                    nc.vector.tensor_add(m01, pw[0], pw[1])
            nc.vector.tensor_add(pw[3], pw[2], pw[3])
            nc.vector.tensor_add(msgq, m01, pw[3])
        # ---- phase D: merge + LayerNorm + store, per quarter -------------
        for q in range(NQ):
            g0 = q * QT
            msgq = msgqs[q]
            srow = mpool.tile([128, NT, 2], FP32, tag="srow")
            sqs = mpool.tile([128, NT, 2], FP32, tag="sqs")
            ysb = []
            for tt in range(NT):
                tok0 = g0 + 128 * tt
                yt = opool.tile([128, D], FP32, tag="ysb")
                ysb.append(yt)
                for half in range(2):
                    n0 = half * 512
                    yps = ps_y.tile([128, 512], FP32, tag="y")
                    for k in range(KD):
                        nc.tensor.matmul(
                            yps,
                            xT_sb[:, k, HALO + tok0:HALO + tok0 + 128],
                            m1_sb[:, k, n0:n0 + 512],
                            start=(k == 0),
                            stop=False,
                        )
                    for k in range(KD):
                        nc.tensor.matmul(
                            yps,
                            msgq[:, k, 128 * tt:128 * tt + 128],
                            m2_sb[:, k, n0:n0 + 512],
                            start=False,
                            stop=(k == KD - 1),
                        )
                    if use_merge_b:
                        nc.vector.tensor_add(yps, yps, mb_sb[:, n0:n0 + 512])
                    # Evict PSUM while collecting LN stats: Copy gives sum(y),
                    # Square gives sum(y^2) — both stay in the sigmoid table
                    # set. 'junk' is a write-only sink for the Square pass.
                    nc.scalar.activation(
                        out=yt[:, n0:n0 + 512], in_=yps, func=AF.Copy,
                        accum_out=srow[:, tt, half:half + 1],
                    )
                    junk = mpool2.tile([128, 512], FP32, tag="junk")
                    nc.scalar.activation(
                        out=junk, in_=yps, func=AF.Square,
                        accum_out=sqs[:, tt, half:half + 1],
                    )
            # LayerNorm finalize for both token tiles at once; rstd via
            # bit-trick seed + 2 Newton steps (keeps sqrt off ScalarE).
            ssum = mpool.tile([128, NT], FP32, tag="ssum")
            nc.vector.reduce_sum(out=ssum, in_=srow, axis=AX.X)
            qsum = mpool.tile([128, NT], FP32, tag="qsum")
            nc.vector.reduce_sum(out=qsum, in_=sqs, axis=AX.X)
            mean = mpool.tile([128, NT], FP32, tag="mean")
            nc.vector.tensor_scalar_mul(mean, ssum, 1.0 / D)
            m2e = mpool.tile([128, NT], FP32, tag="m2e")
            nc.vector.scalar_tensor_tensor(   # mean^2 - eps
                out=m2e, in0=mean, scalar=1.0, in1=mean,
                op0=ALU.mult, op1=ALU.mult,
            )
            nc.vector.tensor_scalar_add(m2e, m2e, -EPS)
            veps = mpool.tile([128, NT], FP32, tag="veps")
            nc.vector.scalar_tensor_tensor(   # q/D - (mean^2 - eps)
                out=veps, in0=qsum, scalar=1.0 / D, in1=m2e,
                op0=ALU.mult, op1=ALU.subtract,
            )
            rbits = mpool.tile([128, NT], I32, tag="rbits")
            nc.vector.tensor_scalar(
                out=rbits, in0=veps.bitcast(I32), scalar1=one_i[:, 0:1],
                scalar2=None, op0=ALU.arith_shift_right,
            )
            nc.vector.tensor_tensor(
                out=rbits, in0=magic_sb.to_broadcast([128, NT]), in1=rbits,
                op=ALU.subtract,
            )
            rstd = rbits.bitcast(FP32)
            for _ in range(2):
                nt1 = mpool.tile([128, NT], FP32, tag="nt1")
                nc.vector.tensor_mul(nt1, rstd, rstd)
                nc.vector.tensor_mul(nt1, nt1, veps)
                nc.vector.tensor_scalar(
                    out=nt1, in0=nt1, scalar1=-0.5, scalar2=1.5,
                    op0=ALU.mult, op1=ALU.add,
                )
                nc.vector.tensor_mul(rstd, rstd, nt1)
            for tt in range(NT):
                tok0 = g0 + 128 * tt
                nc.vector.tensor_scalar(
                    out=ysb[tt], in0=ysb[tt], scalar1=mean[:, tt:tt + 1],
                    scalar2=rstd[:, tt:tt + 1],
                    op0=ALU.subtract, op1=ALU.mult,
                )
                if use_gamma_beta:
                    nc.vector.tensor_mul(ysb[tt], ysb[tt], gam_sb)
                    nc.vector.tensor_add(ysb[tt], ysb[tt], bet_sb)
                nc.sync.dma_start(out=y[tok0:tok0 + 128, :], in_=ysb[tt])
    nc.compile()
    return nc


_CACHE: dict = {}


def _get_nc(use_gamma_beta: bool, use_merge_b: bool):
    key = (use_gamma_beta, use_merge_b)
    if key not in _CACHE:
        _CACHE[key] = build_nc(use_gamma_beta, use_merge_b)
    return _CACHE[key]


def kernel(x, w1, b1, w2, b2, wv_w, wv_b, merge_w, merge_b, gamma, beta):
    x = np.asarray(x, dtype=np.float32)
    w1 = np.asarray(w1, dtype=np.float32)
    b1 = np.asarray(b1, dtype=np.float32)
    w2 = np.asarray(w2, dtype=np.float32)
    b2 = np.asarray(b2, dtype=np.float32)
    wv_w = np.asarray(wv_w, dtype=np.float32)
    wv_b = np.asarray(wv_b, dtype=np.float32)
    merge_w = np.asarray(merge_w, dtype=np.float32)
    merge_b = np.asarray(merge_b, dtype=np.float32)
    gamma = np.asarray(gamma, dtype=np.float32)
    beta = np.asarray(beta, dtype=np.float32)

    use_gamma_beta = not (np.all(gamma == 1.0) and np.all(beta == 0.0))
    use_merge_b = bool(np.any(merge_b != 0.0))
    nc = _get_nc(use_gamma_beta, use_merge_b)

    x2 = x.reshape(B * T, D)
    shared = {
        "w1a": w1[:D].astype(BF16),
        "w1c": w1[D:].astype(BF16),
        "wv": wv_w.astype(BF16),
        "m1": merge_w[:D].astype(BF16),
        "m2": merge_w[D:].astype(BF16),
        "w2rep": np.ascontiguousarray(
            np.broadcast_to(w2.reshape(H, 1), (H, 128))
        ).astype(BF16),
        "b1r": np.ascontiguousarray(b1.reshape(MH, 128).T),
        "wvbr": np.ascontiguousarray(wv_b.reshape(MD, 128).T),
        "b2r": np.full((128, 1), float(b2[0]), np.float32),
    }
    if use_gamma_beta:
        shared["gam"] = gamma.reshape(1, D)
        shared["bet"] = beta.reshape(1, D)
    if use_merge_b:
        shared["mbt"] = merge_b.reshape(1, D)

    in_maps = []
    for c in range(NCORES):
        t0 = c * NTOK
        xs = np.zeros((GRID, D), np.float32)
        xs[HALO:] = x2[t0:t0 + NTOK]
        if t0 % T != 0:  # halo stays inside the same batch element
            xs[:HALO] = x2[t0 - HALO:t0]
        m = dict(shared)
        m["xT"] = np.ascontiguousarray(xs.T).astype(BF16)
        in_maps.append(m)

    res = run_bass_kernel_spmd(nc, in_maps, core_ids=list(range(NCORES)))
    out = np.concatenate([r["y"] for r in res.results], axis=0)
    return out.reshape(B, T, D).astype(np.float32)



# revision 8
# speedup vs baseline: 1.8775x; 1.8775x over previous
"""Trainium2 Bass kernel for CausalTensionGraphLayer.

Math (u-fused factorization; W = 4, H = D/2):
  a   = x @ w1[:D] + b1                        [T, H]
  c   = x @ w1[D:]                             [T, H]   (shifted grid)
  u   = x @ (wv_w @ merge_w[D:]) + wv_b @ merge_w[D:]   [T, D] (shifted grid)
  hid_w  = silu(a[t] + c[t-w-1])
  tau_w  = sigmoid(hid_w @ w2 + b2) = 0.5 + 0.5*tanh(0.5*(...))
  mm[t]  = sum_w tau_w[t] * u[t-w-1]
  y      = x @ merge_w[:D] + mm + merge_b
  out    = LayerNorm(y) * gamma + beta

Fusing wv into merge_w[D:] on the host removes the whole msg @ m2 matmul
(the transposed mm is accumulated into the y PSUM with cheap 128-col
identity matmuls instead) and removes one 2.1 MB weight load.

The gating path (a, c, tau logits) runs in fp8-e4m3 with DoubleRow
matmuls (2x PE throughput); weights are pre-scaled by 32 on the host to
stay out of the fp8 subnormal range and the 1/32 is folded into the PSUM
eviction scale.  The value/merge path stays bf16 (it feeds y directly).
Measured end-to-end rel err ~5e-3 vs the fp32 reference.

Sharding: data-parallel over the B*T = 8192 token rows, 1024 own tokens
per core plus a 4-row halo (zeros at batch boundaries).  No collectives.

All device inputs are host-packed into the exact SBUF layout so every
input DMA is 128 fully contiguous rows (descriptor generation serialized
the old input stage).  The activation table is silu_and_others, so one
ACT_TABLE_LOAD serves the whole kernel; tau's sigmoid is computed as
0.5 + 0.5*tanh(x/2) with the affine folded into a cheap DVE
tensor_scalar.

Emission order software-pipelines quarters: AB0 AB1 C0 AB2 C1 D0 AB3 C2
D1 C3 D2 D3, so the PE stream never waits on the vector/scalar gating
chain.
"""

from contextlib import ExitStack

import numpy as np
import ml_dtypes

import concourse.bass as bass
import concourse.bacc as bacc
import concourse.tile as tile
from concourse import mybir
from concourse.bass_utils import run_bass_kernel_spmd

BF16 = ml_dtypes.bfloat16
F8 = ml_dtypes.float8_e4m3fn
W8SCALE = 32.0

B, T, D = 2, 4096, 1024
H = D // 2
W = 4
EPS = 1e-5
NCORES = 8
NTOK = (B * T) // NCORES          # 1024 own tokens per core
HALO = W                          # 4
GRID = NTOK + HALO                # 1028
NQ = 4                            # token quarters per core
QT = NTOK // NQ                   # 256 own tokens per quarter
QG = QT + HALO                    # 260 shifted-grid cols per quarter
KD = D // 128                     # 8 K-chunks over D
MH = H // 128                     # 4 M-tiles over H
MD = D // 128                     # 8 M-tiles over D
NT = QT // 128                    # 2 token tiles per quarter
G0 = 520                          # xT0 covers grid [0, 520), xT1 [512, 1028)
G1 = GRID - 512                   # 516

FP32 = mybir.dt.float32
I32 = mybir.dt.int32
BF = mybir.dt.bfloat16
E4 = mybir.dt.float8e4
AF = mybir.ActivationFunctionType
ALU = mybir.AluOpType
AX = mybir.AxisListType
DR = mybir.MatmulPerfMode.DoubleRow


def build_nc(use_gamma_beta, use_merge_b, use_b1, use_b2, use_ub):
    nc = bacc.Bacc(None, target_bir_lowering=False)

    xT0 = nc.dram_tensor("xT0", [128, KD * G0], BF, kind="ExternalInput")
    xT1 = nc.dram_tensor("xT1", [128, KD * G1], BF, kind="ExternalInput")
    x80 = nc.dram_tensor("x80", [128, KD * G0], E4, kind="ExternalInput")
    x81 = nc.dram_tensor("x81", [128, KD * G1], E4, kind="ExternalInput")
    w1a8 = nc.dram_tensor("w1a8", [128, KD * H], E4, kind="ExternalInput")
    w1c8 = nc.dram_tensor("w1c8", [128, KD * H], E4, kind="ExternalInput")
    wfA = nc.dram_tensor("wfA", [128, KD * 512], BF, kind="ExternalInput")
    wfB = nc.dram_tensor("wfB", [128, KD * 512], BF, kind="ExternalInput")
    m1h0 = nc.dram_tensor("m1h0", [128, KD * 512], BF, kind="ExternalInput")
    m1h1 = nc.dram_tensor("m1h1", [128, KD * 512], BF, kind="ExternalInput")
    w2r8 = nc.dram_tensor("w2r8", [128, MH * 128], E4, kind="ExternalInput")
    idd = nc.dram_tensor("idd", [128, 128], BF, kind="ExternalInput")
    if use_b1:
        b1r = nc.dram_tensor("b1r", [128, MH], FP32, kind="ExternalInput")
    if use_ub:
        ubr = nc.dram_tensor("ubr", [128, MD], FP32, kind="ExternalInput")
    if use_b2:
        b2h = nc.dram_tensor("b2h", [128, 1], FP32, kind="ExternalInput")
    if use_gamma_beta:
        gam = nc.dram_tensor("gam", [1, D], FP32, kind="ExternalInput")
        bet = nc.dram_tensor("bet", [1, D], FP32, kind="ExternalInput")
    if use_merge_b:
        mbt = nc.dram_tensor("mbt", [1, D], FP32, kind="ExternalInput")
    y = nc.dram_tensor("y", [NTOK, D], BF, kind="ExternalOutput")

    with tile.TileContext(nc) as tc, ExitStack() as ctx:
        persist = ctx.enter_context(tc.tile_pool(name="persist", bufs=1))
        abpool = ctx.enter_context(tc.tile_pool(name="abpool", bufs=NQ))
        qpool = ctx.enter_context(tc.tile_pool(name="qpool", bufs=2))
        mpool = ctx.enter_context(tc.tile_pool(name="mpool", bufs=4))
        mpool2 = ctx.enter_context(tc.tile_pool(name="mpool2", bufs=2))
        opool = ctx.enter_context(tc.tile_pool(name="opool", bufs=5))
        ps_acc = ctx.enter_context(tc.tile_pool(name="ps_acc", bufs=3, space="PSUM"))
        ps_log = ctx.enter_context(tc.tile_pool(name="ps_log", bufs=2, space="PSUM"))
        ps_y = ctx.enter_context(tc.tile_pool(name="ps_y", bufs=3, space="PSUM"))

        # ---- persistent SBUF tiles -------------------------------------
        xT0_sb = persist.tile([128, KD, G0], BF, tag="xT0")
        xT1_sb = persist.tile([128, KD, G1], BF, tag="xT1")
        x80_sb = persist.tile([128, KD, G0], E4, tag="x80")
        x81_sb = persist.tile([128, KD, G1], E4, tag="x81")
        w1a_sb = persist.tile([128, KD, H], E4, tag="w1a8")
        w1c_sb = persist.tile([128, KD, H], E4, tag="w1c8")
        wf_sb = persist.tile([128, KD, D], BF, tag="wf")
        m1h0_sb = persist.tile([128, KD, 512], BF, tag="m1h0")
        m1h1_sb = persist.tile([128, KD, 512], BF, tag="m1h1")
        m1_sb = [m1h0_sb, m1h1_sb]
        w2r_sb = persist.tile([128, MH, 128], E4, tag="w2r8")
        id_sb = persist.tile([128, 128], BF, tag="idd")

        # ---- input DMAs: all host-packed contiguous [128, bytes] -------
        # scalar queue feeds the fp8 gating path (phase A starts on it);
        # sync queue feeds the bf16 value/merge path.
        nc.scalar.dma_start(out=x80_sb, in_=x80[:, :])
        nc.scalar.dma_start(out=w1a_sb, in_=w1a8[:, :])
        nc.scalar.dma_start(out=w1c_sb, in_=w1c8[:, :])
        nc.scalar.dma_start(out=x81_sb, in_=x81[:, :])
        nc.scalar.dma_start(out=w2r_sb, in_=w2r8[:, :])
        nc.scalar.dma_start(out=id_sb, in_=idd[:, :])
        if use_b1:
            b1_sb = persist.tile([128, MH], FP32, tag="b1")
            nc.scalar.dma_start(out=b1_sb, in_=b1r[:, :])
        if use_ub:
            ub_sb = persist.tile([128, MD], FP32, tag="ub")
            nc.scalar.dma_start(out=ub_sb, in_=ubr[:, :])
        if use_b2:
            b2_sb = persist.tile([128, 1], FP32, tag="b2")
            nc.scalar.dma_start(out=b2_sb, in_=b2h[:, :])
        nc.scalar.dma_start(out=m1_sb[0], in_=m1h0[:, :])
        nc.sync.dma_start(out=xT0_sb, in_=xT0[:, :])
        nc.sync.dma_start(out=wf_sb[:, :, 0:512], in_=wfA[:, :])
        nc.sync.dma_start(out=xT1_sb, in_=xT1[:, :])
        nc.sync.dma_start(out=wf_sb[:, :, 512:D], in_=wfB[:, :])
        nc.sync.dma_start(out=m1_sb[1], in_=m1h1[:, :])
        if use_gamma_beta:
            gam_sb = persist.tile([128, D], FP32, tag="gam")
            nc.sync.dma_start(out=gam_sb, in_=gam.partition_broadcast(128))
            bet_sb = persist.tile([128, D], FP32, tag="bet")
            nc.sync.dma_start(out=bet_sb, in_=bet.partition_broadcast(128))
        if use_merge_b:
            mb_sb = persist.tile([128, D], FP32, tag="mb")
            nc.sync.dma_start(out=mb_sb, in_=mbt.partition_broadcast(128))

        magic_sb = persist.tile([128, 1], I32, tag="magic")
        nc.vector.memset(magic_sb, 0x5F3759DF)
        one_i = persist.tile([128, 1], I32, tag="onei")
        nc.vector.memset(one_i, 1)

        # quarter -> (bf16 x tile, fp8 x tile, shifted-grid base col)
        gmap = [
            (xT0_sb, x80_sb, 0), (xT0_sb, x80_sb, 256),
            (xT1_sb, x81_sb, 0), (xT1_sb, x81_sb, 256),
        ]
        ISCALE = 1.0 / W8SCALE

        aqs, cqs, uqs, tauqs, mmqs = {}, {}, {}, {}, {}

        def emit_AB(q):
            xs, x8, base = gmap[q]
            aq = abpool.tile([128, MH, QT], BF, tag="aq")
            cq = abpool.tile([128, MH, QG], BF, tag="cq")
            uq = abpool.tile([128, MD, QG], BF, tag="uq")
            aqs[q], cqs[q], uqs[q] = aq, cq, uq
            for m in range(MH):
                ps = ps_acc.tile([128, QG], FP32, tag="acc")
                for kp in range(KD // 2):
                    nc.tensor.matmul(
                        ps[:, 0:QT],
                        w1a_sb[:, 2 * kp:2 * kp + 2, m * 128:(m + 1) * 128],
                        x8[:, 2 * kp:2 * kp + 2, base + HALO:base + HALO + QT],
                        start=(kp == 0), stop=(kp == KD // 2 - 1),
                        perf_mode=DR,
                    )
                if use_b1:
                    nc.scalar.activation(
                        out=aq[:, m, :], in_=ps[:, 0:QT], func=AF.Identity,
                        bias=b1_sb[:, m:m + 1], scale=ISCALE,
                    )
                elif m % 2 == 0:
                    nc.scalar.activation(
                        out=aq[:, m, :], in_=ps[:, 0:QT], func=AF.Identity,
                        bias=0.0, scale=ISCALE,
                    )
                else:
                    nc.vector.tensor_scalar_mul(aq[:, m, :], ps[:, 0:QT], ISCALE)
            for m in range(MH):
                ps = ps_acc.tile([128, QG], FP32, tag="acc")
                for kp in range(KD // 2):
                    nc.tensor.matmul(
                        ps,
                        w1c_sb[:, 2 * kp:2 * kp + 2, m * 128:(m + 1) * 128],
                        x8[:, 2 * kp:2 * kp + 2, base:base + QG],
                        start=(kp == 0), stop=(kp == KD // 2 - 1),
                        perf_mode=DR,
                    )
                if m % 2 == 0:
                    nc.scalar.activation(
                        out=cq[:, m, :], in_=ps, func=AF.Identity,
                        bias=0.0, scale=ISCALE,
                    )
                else:
                    nc.vector.tensor_scalar_mul(cq[:, m, :], ps, ISCALE)
            for m in range(MD):
                ps = ps_acc.tile([128, QG], FP32, tag="acc")
                for k in range(KD):
                    nc.tensor.matmul(
                        ps, wf_sb[:, k, m * 128:(m + 1) * 128],
                        xs[:, k, base:base + QG],
                        start=(k == 0), stop=(k == KD - 1),
                    )
                if use_ub:
                    nc.scalar.activation(
                        out=uq[:, m, :], in_=ps, func=AF.Identity,
                        bias=ub_sb[:, m:m + 1], scale=1.0,
                    )
                else:
                    nc.scalar.copy(out=uq[:, m, :], in_=ps)

        def emit_C(q):
            aq, cq, uq = aqs[q], cqs[q], uqs[q]
            tauq = qpool.tile([128, W, QT], BF, tag="tauq")
            tauqs[q] = tauq

            def tau_b(w):
                s = tauq[:, w, :]
                return bass.AP(
                    tensor=s.tensor, offset=s.offset,
                    ap=[s.ap[0], [0, MD], s.ap[1]],
                )

            pw = {}
            m01 = None
            for p in range(W // 2):
                hs = mpool2.tile([128, MH, 2, QT], BF, tag="hs")
                for wi in range(2):
                    w = 2 * p + wi
                    o = HALO - 1 - w
                    nc.vector.tensor_add(hs[:, :, wi, :], aq, cq[:, :, o:o + QT])
                hss = mpool2.tile([128, MH, 2, QT], E4, tag="hss")
                nc.scalar.activation(out=hss, in_=hs, func=AF.Silu)
                pl = ps_log.tile([128, 2 * QT], FP32, tag="logit")
                for kp in range(MH // 2):
                    nc.tensor.matmul(
                        pl, w2r_sb[:, 2 * kp:2 * kp + 2, :],
                        hss[:, 2 * kp:2 * kp + 2, :, :],
                        start=(kp == 0), stop=(kp == MH // 2 - 1),
                        perf_mode=DR,
                    )
                # tau = 0.5 + 0.5*tanh(0.5*(logit + b2)); affine done on DVE
                nc.scalar.activation(
                    out=tauq[:, 2 * p:2 * p + 2, :],
                    in_=pl.rearrange("p (a b) -> p a b", a=2),
                    func=AF.Tanh,
                    bias=(b2_sb[:, 0:1] if use_b2 else 0.0),
                    scale=0.5 * ISCALE,
                )
                nc.vector.tensor_scalar(
                    out=tauq[:, 2 * p:2 * p + 2, :],
                    in0=tauq[:, 2 * p:2 * p + 2, :],
                    scalar1=0.5, scalar2=0.5, op0=ALU.mult, op1=ALU.add,
                )
                for wi in range(2):
                    w = 2 * p + wi
                    o = HALO - 1 - w
                    pt = mpool.tile([128, MD, QT], BF, tag="pw")
                    nc.vector.tensor_mul(pt, tau_b(w), uq[:, :, o:o + QT])
                    pw[w] = pt
                if p == 0:
                    m01 = mpool.tile([128, MD, QT], BF, tag="pw")
                    nc.vector.tensor_add(m01, pw[0], pw[1])
            mmq = qpool.tile([128, MD, QT], BF, tag="mmq")
            mmqs[q] = mmq
            nc.vector.tensor_add(pw[3], pw[2], pw[3])
            nc.vector.tensor_add(mmq, m01, pw[3])

        def emit_D(q):
            xs, _, base = gmap[q]
            mmq = mmqs[q]
            srow = mpool.tile([128, NT, 2], FP32, tag="srow")
            sqs = mpool.tile([128, NT, 2], FP32, tag="sqs")
            ysb = []
            for tt in range(NT):
                tcol = base + HALO + tt * 128
                yt = opool.tile([128, D], BF, tag="ysb")
                ysb.append(yt)
                yp0 = ps_y.tile([128, 512], FP32, tag="y")
                yp1 = ps_y.tile([128, 512], FP32, tag="y")
                yps = [yp0, yp1]
                for k in range(KD):
                    for half in range(2):
                        nc.tensor.matmul(
                            yps[half], xs[:, k, tcol:tcol + 128],
                            m1_sb[half][:, k, :],
                            start=(k == 0), stop=False,
                        )
                for m in range(MD):
                    half, j0 = m // 4, (m % 4) * 128
                    nc.tensor.matmul(
                        yps[half][:, j0:j0 + 128],
                        mmq[:, m, tt * 128:tt * 128 + 128], id_sb,
                        start=False, stop=(m % 4 == 3), skip_group_check=True,
                    )
                for half in range(2):
                    n0 = half * 512
                    if use_merge_b:
                        nc.vector.tensor_add(
                            yps[half], yps[half], mb_sb[:, n0:n0 + 512]
                        )
                    nc.scalar.activation(
                        out=yt[:, n0:n0 + 512], in_=yps[half], func=AF.Copy,
                        accum_out=srow[:, tt, half:half + 1],
                    )
                    y2 = mpool2.tile([128, 512], BF, tag="y2")
                    nc.vector.scalar_tensor_tensor(
                        out=y2, in0=yt[:, n0:n0 + 512], scalar=1.0,
                        in1=yt[:, n0:n0 + 512], op0=ALU.mult, op1=ALU.mult,
                        accum_out=sqs[:, tt, half:half + 1],
                    )
            # LayerNorm stats; rstd via bit-trick seed + 1 Newton step
            ssum = mpool.tile([128, NT], FP32, tag="ssum")
            nc.vector.reduce_sum(out=ssum, in_=srow, axis=AX.X)
            qsum = mpool.tile([128, NT], FP32, tag="qsum")
            nc.vector.reduce_sum(out=qsum, in_=sqs, axis=AX.X)
            mean = mpool.tile([128, NT], FP32, tag="mean")
            nc.vector.tensor_scalar_mul(mean, ssum, 1.0 / D)
            m2e = mpool.tile([128, NT], FP32, tag="m2e")
            nc.vector.scalar_tensor_tensor(   # mean^2 - eps
                out=m2e, in0=mean, scalar=1.0, in1=mean,
                op0=ALU.mult, op1=ALU.mult,
            )
            nc.vector.tensor_scalar_add(m2e, m2e, -EPS)
            veps = mpool.tile([128, NT], FP32, tag="veps")
            nc.vector.scalar_tensor_tensor(   # q/D - (mean^2 - eps)
                out=veps, in0=qsum, scalar=1.0 / D, in1=m2e,
                op0=ALU.mult, op1=ALU.subtract,
            )
            rbits = mpool.tile([128, NT], I32, tag="rbits")
            nc.vector.tensor_scalar(
                out=rbits, in0=veps.bitcast(I32), scalar1=one_i[:, 0:1],
                scalar2=None, op0=ALU.arith_shift_right,
            )
            nc.vector.tensor_tensor(
                out=rbits, in0=magic_sb.to_broadcast([128, NT]), in1=rbits,
                op=ALU.subtract,
            )
            rstd = rbits.bitcast(FP32)
            for _ in range(1):
                nt1 = mpool.tile([128, NT], FP32, tag="nt1")
                nc.vector.tensor_mul(nt1, rstd, rstd)
                nc.vector.tensor_mul(nt1, nt1, veps)
                nc.vector.tensor_scalar(
                    out=nt1, in0=nt1, scalar1=-0.5, scalar2=1.5,
                    op0=ALU.mult, op1=ALU.add,
                )
                nc.vector.tensor_mul(rstd, rstd, nt1)
            for tt in range(NT):
                tok0 = q * QT + 128 * tt
                nc.vector.tensor_scalar(
                    out=ysb[tt], in0=ysb[tt], scalar1=mean[:, tt:tt + 1],
                    scalar2=rstd[:, tt:tt + 1],
                    op0=ALU.subtract, op1=ALU.mult,
                )
                if use_gamma_beta:
                    nc.vector.tensor_mul(ysb[tt], ysb[tt], gam_sb)
                    nc.vector.tensor_add(ysb[tt], ysb[tt], bet_sb)
                nc.sync.dma_start(out=y[tok0:tok0 + 128, :], in_=ysb[tt])

        # software-pipelined emission: D lags C by one quarter
        emit_AB(0)
        emit_AB(1)
        emit_C(0)
        emit_AB(2)
        emit_C(1)
        emit_D(0)
        emit_AB(3)
        emit_C(2)
        emit_D(1)
        emit_C(3)
        emit_D(2)
        emit_D(3)
    nc.compile()
    return nc


_CACHE: dict = {}


def _get_nc(*flags):
    if flags not in _CACHE:
        _CACHE[flags] = build_nc(*flags)
    return _CACHE[flags]


def _pack(a):
    # [D, F] -> [128, KD*F] in the SBUF layout (partition = d % 128 within
    # each 128-row K-chunk)
    d, f = a.shape
    return np.ascontiguousarray(
        a.reshape(d // 128, 128, f).transpose(1, 0, 2).reshape(128, -1)
    )


def kernel(x, w1, b1, w2, b2, wv_w, wv_b, merge_w, merge_b, gamma, beta):
    x = np.asarray(x, dtype=np.float32)
    w1 = np.asarray(w1, dtype=np.float32)
    b1 = np.asarray(b1, dtype=np.float32)
    w2 = np.asarray(w2, dtype=np.float32)
    b2 = np.asarray(b2, dtype=np.float32)
    wv_w = np.asarray(wv_w, dtype=np.float32)
    wv_b = np.asarray(wv_b, dtype=np.float32)
    merge_w = np.asarray(merge_w, dtype=np.float32)
    merge_b = np.asarray(merge_b, dtype=np.float32)
    gamma = np.asarray(gamma, dtype=np.float32)
    beta = np.asarray(beta, dtype=np.float32)

    m2 = merge_w[D:]
    wfuse = wv_w @ m2
    ubias = wv_b @ m2
    use_gamma_beta = not (np.all(gamma == 1.0) and np.all(beta == 0.0))
    use_merge_b = bool(np.any(merge_b != 0.0))
    use_b1 = bool(np.any(b1 != 0.0))
    use_b2 = bool(np.any(b2 != 0.0))
    use_ub = bool(np.any(ubias != 0.0))
    nc = _get_nc(use_gamma_beta, use_merge_b, use_b1, use_b2, use_ub)

    shared = {
        "w1a8": _pack((W8SCALE * w1[:D]).astype(F8)),
        "w1c8": _pack((W8SCALE * w1[D:]).astype(F8)),
        "wfA": _pack(wfuse[:, 0:512].astype(BF16)),
        "wfB": _pack(wfuse[:, 512:D].astype(BF16)),
        "m1h0": _pack(merge_w[:D, 0:512].astype(BF16)),
        "m1h1": _pack(merge_w[:D, 512:D].astype(BF16)),
        "w2r8": _pack(
            np.ascontiguousarray(
                np.broadcast_to(
                    (W8SCALE * w2).reshape(H, 1), (H, 128)
                )
            ).astype(F8)
        ),
        "idd": np.eye(128, dtype=np.float32).astype(BF16),
    }
    if use_b1:
        shared["b1r"] = np.ascontiguousarray(b1.reshape(MH, 128).T)
    if use_ub:
        shared["ubr"] = np.ascontiguousarray(ubias.reshape(MD, 128).T)
    if use_b2:
        shared["b2h"] = np.full((128, 1), 0.5 * float(b2[0]), np.float32)
    if use_gamma_beta:
        shared["gam"] = gamma.reshape(1, D)
        shared["bet"] = beta.reshape(1, D)
    if use_merge_b:
        shared["mbt"] = merge_b.reshape(1, D)

    x2 = x.reshape(B * T, D)
    in_maps = []
    for c in range(NCORES):
        t0 = c * NTOK
        xs = np.zeros((GRID, D), np.float32)
        xs[HALO:] = x2[t0:t0 + NTOK]
        if t0 % T != 0:  # halo stays inside the same batch element
            xs[:HALO] = x2[t0 - HALO:t0]
        xt = np.ascontiguousarray(xs.T).astype(BF16)
        x8full = xt.astype(np.float32).astype(F8)
        m = dict(shared)
        m["xT0"] = _pack(xt[:, 0:G0])
        m["xT1"] = _pack(xt[:, 512:GRID])
        m["x80"] = _pack(x8full[:, 0:G0])
        m["x81"] = _pack(x8full[:, 512:GRID])
        in_maps.append(m)

    res = run_bass_kernel_spmd(nc, in_maps, core_ids=list(range(NCORES)))
    out = np.concatenate(
        [r["y"].astype(np.float32) for r in res.results], axis=0
    )
    return out.reshape(B, T, D)
